# revision 42
# baseline (speedup 1.0000x reference)
"""AggregateGCN (3x GraphConv norm='both' + mean-pool + linear) on 8 trn2 cores.

Inspector-executor design:
  * Nodes permuted by in-degree rank, dst-sharded round-robin across cores
    (balanced edge counts, degree-sorted locals). Core owns L locals
    (N/8 real + pad) = NB blocks of 128.
  * y = (x * out_norm) @ W computed on the local shard, AllGathered into a
    per-core DRAM replica padded to 64-f32 rows (256B gather granularity).
  * Message passing: dst-sorted per-core edge stream gathered with the SWDGE
    dma_gather custom op (int16 idx -> NCH src-chunk passes). Gathers STREAM
    back-to-back (ring reclaim works; measured 2.36 ns/idx vs 4.05 with
    drain cycles); the only gating is s_g buffer reuse via PE progress.
  * Segment-sum: one-hot dst indicators built 8 tiles per DVE is_equal
    (3D broadcast AP, iota-vs-dstslot) feed per-tile PE matmuls
    accumulating into PSUM-resident agg [128 x NB*32].
  * Pooling: per-block PE matmul vs host one-hot, AllReduce, 1/count, We+be.
"""

from dataclasses import dataclass

import numpy as np


@dataclass(frozen=True)
class Cfg:
    N: int = 100_000
    E: int = 3_200_000
    G: int = 64
    IN_DIM: int = 128
    H: int = 32
    EMB: int = 16
    CORES: int = 8
    NCH: int = 4
    P: int = 128
    USE_BF16: bool = True
    INST_IDX: int = 1024
    NQ: int = 4
    IND_RING: int = 32
    HALF: int = 8

    @property
    def ELEM(self):          # gather row width in elements (256B granularity)
        return 128 if self.USE_BF16 else 64

    @property
    def L(self):
        per = -(-self.N // self.CORES)
        return -(-per // self.P) * self.P

    @property
    def NB(self):
        return self.L // self.P

    @property
    def NPAD(self):
        return self.CORES * self.L

    @property
    def CHROWS(self):
        assert self.NPAD % self.NCH == 0
        return self.NPAD // self.NCH

    @property
    def TPI(self):
        return self.INST_IDX // self.P

    @property
    def CYC_TILES(self):
        return self.NQ * self.TPI


CFG = Cfg()


def _owner_is_dve(t):
    return True                 # all indicator tiles on DVE


# ===================================================================== host
def _preprocess(cfg, h, src, dst, graph_ids):
    N, G = cfg.N, cfg.G
    P, L, NB, CORES, NCH = cfg.P, cfg.L, cfg.NB, cfg.CORES, cfg.NCH
    CHROWS, TPI, CYC_TILES = cfg.CHROWS, cfg.TPI, cfg.CYC_TILES

    deg_in = np.bincount(dst, minlength=N).astype(np.int64)
    deg_out = np.bincount(src, minlength=N).astype(np.int64)

    order = np.argsort(-deg_in, kind="stable")
    rank_of = np.empty(N, dtype=np.int64)
    rank_of[order] = np.arange(N)
    new_of = (rank_of % CORES) * L + rank_of // CORES

    src_new = new_of[src]
    dst_new = new_of[dst]
    dcore = dst_new // L
    dslot = (dst_new % L) % P
    dblk = (dst_new % L) // P
    qchunk = src_new // CHROWS
    sidx = src_new % CHROWS

    key = (dcore * NCH + qchunk) * NB + dblk
    counts = np.bincount(key, minlength=CORES * NCH * NB)
    tiles_bq = np.maximum(
        -(-counts.reshape(CORES, NCH, NB).max(axis=0) // P), 1)

    tile_block, tile_pass = [], []
    seg_tile0 = np.zeros((NCH, NB), dtype=np.int64)
    for q in range(NCH):
        for b in range(NB):
            seg_tile0[q, b] = len(tile_block)
            tile_block += [b] * int(tiles_bq[q, b])
            tile_pass += [q] * int(tiles_bq[q, b])
        while len(tile_block) % TPI:
            tile_block.append(-1)
            tile_pass.append(q)
    while len(tile_block) % CYC_TILES:
        tile_block.append(-1)
        tile_pass.append(NCH - 1)
    tile_block = np.array(tile_block, dtype=np.int64)
    tile_pass = np.array(tile_pass, dtype=np.int64)
    T = len(tile_block)

    # start/stop at PSUM-bank granularity (zero region = 2KB = 16 blocks)
    start_f = np.zeros(T, dtype=bool)
    stop_f = np.zeros(T, dtype=bool)
    bank_of = tile_block // 16
    for k in range(int(bank_of.max()) + 1):
        w = np.nonzero((bank_of == k) & (tile_block >= 0))[0]
        if len(w):
            start_f[w[0]] = True
            stop_f[w[-1]] = True

    ptr = np.zeros(CORES * NCH * NB + 1, dtype=np.int64)
    np.add.at(ptr[1:], key, 1)
    np.cumsum(ptr, out=ptr)
    edges_sorted = np.argsort(key, kind="stable")

    S = T * P
    NINST = T // TPI
    zrow = L - 1
    per_core = []
    for c in range(CORES):
        idx16 = np.full(S, zrow, dtype=np.int64)
        dsl = np.full(S, -1.0, dtype=np.float32)
        for q in range(NCH):
            base_k = (c * NCH + q) * NB
            for b in range(NB):
                es = edges_sorted[ptr[base_k + b]:ptr[base_k + b + 1]]
                base = seg_tile0[q, b] * P
                idx16[base:base + len(es)] = sidx[es]
                dsl[base:base + len(es)] = dslot[es]
        w = idx16.reshape(NINST, cfg.INST_IDX // 16, 16).transpose(0, 2, 1)
        wrapped = np.tile(w, (1, 8, 1))
        idx_arr = np.ascontiguousarray(
            wrapped.transpose(1, 0, 2).reshape(P, -1)).astype(np.int16)
        per_core.append({
            "idx16": idx_arr,
            "dstneg": np.ascontiguousarray(
                (-dsl.reshape(T, P).T).astype(np.float32)),
        })

    deg_in_c = np.maximum(deg_in.astype(np.float32), 1.0)
    deg_out_c = np.maximum(deg_out.astype(np.float32), 1.0)
    for c in range(CORES):
        olds = order[np.arange(c, N, CORES)]
        nreal = len(olds)
        hs = np.zeros((L, cfg.IN_DIM), dtype=np.float32)
        hs[:nreal] = h[olds]
        dit = np.full(L, 1e30, dtype=np.float32)
        dot = np.full(L, 1e30, dtype=np.float32)
        dit[:nreal] = deg_in_c[olds]
        dot[:nreal] = deg_out_c[olds]
        po = np.zeros((L, G), dtype=np.float32)
        po[np.arange(nreal), graph_ids[olds]] = 1.0
        d = per_core[c]
        d["h_shard"] = hs
        d["deg_in_t"] = np.ascontiguousarray(dit.reshape(NB, P).T)
        d["deg_out_t"] = np.ascontiguousarray(dot.reshape(NB, P).T)
        d["pool1h"] = np.ascontiguousarray(po.reshape(NB, P, G))

    gcnt = np.bincount(graph_ids, minlength=G).astype(np.float32)
    common = {
        "iota_pos": np.broadcast_to(
            np.arange(P, dtype=np.float32), (P, P)).copy(),
        "iota_neg": np.broadcast_to(
            -np.arange(P, dtype=np.float32), (P, P)).copy(),
        "ident": np.eye(P, dtype=np.float32),
        "counts_rep": np.broadcast_to(gcnt, (cfg.H, G)).copy(),
    }
    meta = dict(T=T, NINST=NINST, NCYC=T // CYC_TILES,
                tile_block=tile_block, start_f=start_f, stop_f=stop_f,
                inst_pass=tile_pass[::TPI].copy())
    return per_core, common, meta


# ==================================================================== device
def _build(cfg, meta):
    from contextlib import ExitStack

    from concourse import bacc, mybir

    f32 = mybir.dt.float32
    i16 = mybir.dt.int16
    gdt = mybir.dt.bfloat16 if cfg.USE_BF16 else f32
    AF = mybir.ActivationFunctionType
    OP = mybir.AluOpType

    P, L, NB, G, H, EMB = cfg.P, cfg.L, cfg.NB, cfg.G, cfg.H, cfg.EMB
    IN_DIM, ELEM, NCH, NQ = cfg.IN_DIM, cfg.ELEM, cfg.NCH, cfg.NQ
    CHROWS, INST_IDX, TPI = cfg.CHROWS, cfg.INST_IDX, cfg.TPI
    CYC_TILES, IND_RING, HALF = cfg.CYC_TILES, cfg.IND_RING, cfg.HALF
    T, NINST, NCYC = meta["T"], meta["NINST"], meta["NCYC"]
    tile_block = meta["tile_block"]
    start_f, stop_f = meta["start_f"], meta["stop_f"]
    inst_pass = meta["inst_pass"]
    SIDX = NINST * (INST_IDX // 16)
    NGRP = (NB + 3) // 4
    grp_rows = [min(4, NB - 4 * j) for j in range(NGRP)]

    # cumulative per-owner indicator counts (all layers share the pattern)
    owner_d = np.array([_owner_is_dve(t) for t in range(T)])
    cum_d = np.cumsum(owner_d)            # dve count through tile t (1-based)
    cum_a = np.cumsum(~owner_d)
    ND_L, NA_L = int(cum_d[-1]), int(cum_a[-1])   # per-layer owner totals

    # PSUM-bank pipelining: post/relu/transforms start per 16-block bank as
    # soon as its accumulation group stops (during the tail gather chunk).
    NBANK = (NB + 15) // 16
    bank_of_t = np.where(tile_block >= 0, tile_block // 16, -1)
    bank_last_half = np.zeros(NBANK, dtype=np.int64)
    bank_first_cyc = np.zeros(NBANK, dtype=np.int64)
    bank_last_cyc = np.zeros(NBANK, dtype=np.int64)
    for k in range(NBANK):
        w = np.nonzero(bank_of_t == k)[0]
        bank_last_half[k] = w[-1] // HALF + 1
        bank_first_cyc[k] = w[0] // CYC_TILES
        bank_last_cyc[k] = w[-1] // CYC_TILES
    bank_groups = [list(range(4 * k, min(4 * k + 4, NGRP)))
                   for k in range(NBANK)]
    za_after_bank = [min(4 * (k + 1), NGRP) for k in range(NBANK)]

    nc = bacc.Bacc("TRN2", num_swdge_queues=NQ)

    def din(name, shape, dtype=f32):
        return nc.dram_tensor(name, shape, dtype, kind="ExternalInput")

    h_shard = din("h_shard", [L, IN_DIM])
    idx16 = din("idx16", [P, SIDX], i16)
    dstneg = din("dstneg", [P, T])
    deg_in_t = din("deg_in_t", [P, NB])
    deg_out_t = din("deg_out_t", [P, NB])
    pool1h = din("pool1h", [NB, P, G])
    iota_pos = din("iota_pos", [P, P])
    iota_neg = din("iota_neg", [P, P])
    ident = din("ident", [P, P])
    counts_rep = din("counts_rep", [H, G])
    W1 = din("W1", [IN_DIM, H])
    W2 = din("W2", [H, H])
    W3 = din("W3", [H, H])
    We = din("We", [H, EMB])
    b1r = din("b1r", [P, H])
    b2r = din("b2r", [P, H])
    b3r = din("b3r", [P, H])
    ber = din("ber", [EMB, 1])
    outT = nc.dram_tensor("outT", [EMB, G], f32, kind="ExternalOutput")

    y_shard = [nc.dram_tensor(f"y_shard{l}", [L, ELEM], gdt)
               for l in range(3)]
    y_full = [nc.dram_tensor(f"y_full{l}", [cfg.NPAD, ELEM], gdt,
                             addr_space="Shared") for l in range(3)]
    ar_in = nc.dram_tensor("ar_in", [H, G], f32)
    ar_out = nc.dram_tensor("ar_out", [H, G], f32, addr_space="Shared")

    sb = nc.alloc_sbuf_tensor
    s_idx = sb("s_idx", [P, SIDX], i16)
    s_dstn = sb("s_dstn", [P, T], f32)
    s_iop = sb("s_iop", [P, P], f32)
    s_ion = sb("s_ion", [P, P], f32)
    s_id = sb("s_id", [P, P], f32)
    s_cnt = sb("s_cnt", [H, G], f32)
    s_W1 = sb("s_W1", [IN_DIM, H], f32)
    s_Wn = [sb("s_W2", [H, H], f32), sb("s_W3", [H, H], f32)]
    s_We = sb("s_We", [H, EMB], f32)
    s_b = [sb(f"s_bb{l}", [P, H], f32) for l in range(3)]
    s_be = sb("s_be", [EMB, 1], f32)
    s_inn = sb("s_inn", [P, NB], f32)
    s_onn = sb("s_onn", [P, NB], f32)
    s_poall = sb("s_poall", [P, NB * G], f32)
    s_x = sb("s_x", [P, NB * H], f32)
    s_zall = sb("s_zall", [P, NB * H], f32)
    s_h4 = [sb(f"s_h4{i}", [P, 4 * IN_DIM], f32) for i in range(2)]
    s_z4 = [sb(f"s_z4{i}", [P, 4 * IN_DIM], f32) for i in range(2)]
    s_zT4 = [sb(f"s_zT4{i}", [P, 4 * P], f32) for i in range(2)]
    s_zc4 = [sb(f"s_zc4{i}", [H, 4 * P], f32) for i in range(2)]
    s_y4 = [sb(f"s_y4{i}", [P, 4 * ELEM], gdt) for i in range(2)]
    NGBUF = 5
    s_g = [sb(f"s_g{i}", [P, CYC_TILES * ELEM], gdt) for i in range(NGBUF)]
    s_indf = sb("s_indf", [P, IND_RING * P], gdt)
    s_ind = [s_indf[:, i * P:(i + 1) * P] for i in range(IND_RING)]
    IND_B = HALF                       # tiles per DVE indicator build
    RING_G = IND_RING // IND_B         # indicator ring in groups
    NGRP_I = T // IND_B                # indicator groups per layer
    s_pl = sb("s_pl", [H, G], f32)
    s_hg = sb("s_hg", [H, G], f32)
    s_o2 = sb("s_o2", [EMB, G], f32)

    ctx = ExitStack()
    ps = ctx.enter_context(nc.psum_tensor([P, 4096], f32))
    AGB = lambda b: H * b
    AGG = lambda b: ps[:, H * b:H * b + H]
    TRS = [ps[:, 3136:3264], ps[:, 3264:3392],
           ps[:, 3392:3520], ps[:, 3584:3712]]
    YP4 = ps[:, 3712:3840]
    YP = lambda a: ps[:, 3712 + H * a:3712 + H * a + H]
    PLD = ps[0:H, 3840:3840 + G]
    O2 = ps[0:EMB, 3968:3968 + G]

    S = {}
    for name in ["ld", "h4", "z4", "t4", "zc4", "ym4", "y4c", "ywr",
                 "norm", "cc", "indd", "inda",
                 "pe", "xd", "xa", "za", "plm", "plc",
                 "arw", "hgl", "hgm", "o2m", "o2c", "outw",
                 "sact", "sdve", "normr"]:
        S[name] = ctx.enter_context(nc.semaphore(name))
    SG = [[ctx.enter_context(nc.semaphore(f"sg{p}{j}")) for j in range(NQ)]
          for p in range(NGBUF)]

    RG = [list(range(cfg.CORES))]
    NLD = 17 * 16
    # cumulative transposes through group gj (global over transforms)
    g_rows_all = grp_rows * 3
    t4_after = np.cumsum(g_rows_all)

    # python-side cumulative counters for cross-engine waits
    ct = dict(t4=0, zc4=0, ym4=0, y4c=0, ywr=0, z4=0, h4=0,
              sact=0, sdve=0)

    with nc.Block() as block:

        # ---------------- SP ----------------
        @block.sync
        def _(sp):
            loads = [(s_idx, idx16[:]), (s_dstn, dstneg[:]),
                     (s_iop, iota_pos[:]), (s_ion, iota_neg[:]),
                     (s_id, ident[:]), (s_cnt, counts_rep[:]),
                     (s_W1, W1[:]), (s_Wn[0], W2[:]), (s_Wn[1], W3[:]),
                     (s_We, We[:]), (s_b[0], b1r[:]), (s_b[1], b2r[:]),
                     (s_b[2], b3r[:]), (s_be, ber[:]),
                     (s_inn, deg_in_t[:]), (s_onn, deg_out_t[:])]
            for t_, d_ in loads:
                sp.dma_start(t_[:], d_).then_inc(S["ld"], 16)
            sp.dma_start(
                s_poall[:].rearrange("p (b g) -> p b g", g=G),
                pool1h[:].rearrange("b p g -> p b g"),
            ).then_inc(S["ld"], 16)
            for l in range(3):
                for j in range(NGRP):
                    i = j % 2
                    r = grp_rows[j]
                    gj = l * NGRP + j
                    if l == 0:
                        if j >= 2:
                            sp.wait_ge(S["z4"], j - 1)
                        sp.dma_start(
                            s_h4[i][:, :r * IN_DIM]
                            .rearrange("p (a f) -> p a f", f=IN_DIM),
                            h_shard[4 * P * j:4 * P * j + r * P, :]
                            .rearrange("(a p) f -> p a f", p=P),
                        ).then_inc(S["h4"], 16)
                    sp.wait_ge(S["y4c"], gj + 1)
                    sp.dma_start(
                        y_shard[l][4 * P * j:4 * P * j + r * P, :]
                        .rearrange("(a p) f -> p a f", p=P),
                        s_y4[i][:, :r * ELEM]
                        .rearrange("p (a e) -> p a e", e=ELEM),
                    ).then_inc(S["ywr"], 16)
            sp.wait_ge(S["plc"], 1)
            sp.dma_start(ar_in[:], s_pl[:]).then_inc(S["arw"], 16)
            sp.wait_ge(S["cc"], 4)
            sp.dma_start(s_hg[:], ar_out[:]).then_inc(S["hgl"], 16)
            sp.wait_ge(S["o2c"], 1)
            sp.dma_start(outT[:], s_o2[:]).then_inc(S["outw"], 16)
            sp.wait_ge(S["outw"], 16)

        # ---------------- ACT ----------------
        @block.scalar
        def _(act):
            act.wait_ge(S["normr"], 2)
            act.activation(s_inn[:], s_inn[:], AF.Sqrt).then_inc(S["norm"], 1)
            act.activation(s_onn[:], s_onn[:], AF.Sqrt).then_inc(S["norm"], 1)
            for l in range(3):
                act.wait_ge(S["xd"], l + 1)
                act.activation(s_x[:], s_x[:], AF.Relu).then_inc(S["xa"], 1)

        # ---------------- DVE ----------------
        @block.vector
        def _(dve):
            def chain(inst):
                ct["sdve"] += 1
                inst.then_inc(S["sdve"], 1)
                dve.wait_ge(S["sdve"], ct["sdve"])

            dve.wait_ge(S["ld"], NLD)
            dve.memset(s_y4[0][:], 0.0)
            dve.memset(s_y4[1][:], 0.0)
            dve.reciprocal(s_inn[:], s_inn[:]).then_inc(S["normr"], 1)
            dve.reciprocal(s_onn[:], s_onn[:]).then_inc(S["normr"], 1)
            dve.wait_ge(S["norm"], 2)

            def transform_group(tf, j):
                i = j % 2
                r = grp_rows[j]
                gj = tf * NGRP + j
                if tf == 0:
                    dve.wait_ge(S["h4"], 16 * (j + 1))
                    if j >= 2:
                        # s_z4[i] reused: PE transposed group j-2
                        dve.wait_ge(S["t4"], int(t4_after[j - 2]))
                    dve.tensor_tensor(
                        s_z4[i][:, :r * IN_DIM]
                        .rearrange("p (a f) -> p a f", f=IN_DIM),
                        s_h4[i][:, :r * IN_DIM]
                        .rearrange("p (a f) -> p a f", f=IN_DIM),
                        s_onn[:, 4 * j:4 * j + r]
                        .unsqueeze(2).to_broadcast([P, r, IN_DIM]),
                        OP.mult).then_inc(S["z4"], 1)
                hh = IN_DIM if tf == 0 else H
                dst_t = (s_zT4 if tf == 0 else s_zc4)[i]
                dve.wait_ge(S["t4"], int(t4_after[gj]))  # group transposed
                for a in range(r):
                    inst = dve.tensor_copy(dst_t[0:hh, a * P:(a + 1) * P],
                                           TRS[a][0:hh, :])
                inst.then_inc(S["zc4"], 1)
                dve.wait_ge(S["ym4"], int(t4_after[gj]))  # group matmuls done
                if gj >= 2:
                    dve.wait_ge(S["ywr"], 16 * (gj - 1))
                inst = dve.tensor_copy(
                    s_y4[i][:, :r * ELEM]
                    .rearrange("p (a e) -> p a e", e=ELEM)[:, :, 0:H],
                    YP4[:, :r * H].rearrange("p (a f) -> p a f", f=H))
                inst.then_inc(S["y4c"], 1)

            # transform 0 (h -> y0)
            for j in range(NGRP):
                transform_group(0, j)

            for l in range(3):
                for gi in range(NGRP_I):
                    icg = l * NGRP_I + gi
                    if icg >= RING_G:
                        dve.wait_ge(S["pe"], icg - RING_G + 1)
                    dve.tensor_tensor(
                        s_indf[:, (icg % RING_G) * IND_B * P:
                               ((icg % RING_G) + 1) * IND_B * P]
                        .rearrange("p (b q) -> p b q", q=P),
                        s_dstn[:, gi * IND_B:(gi + 1) * IND_B]
                        .unsqueeze(2).to_broadcast([P, IND_B, P]),
                        s_ion[:].unsqueeze(1).to_broadcast([P, IND_B, P]),
                        OP.is_equal).then_inc(S["indd"], 1)
                # post
                dve.wait_ge(S["pe"], (l + 1) * NCYC * (CYC_TILES // HALF))
                chain(dve.tensor_tensor(
                    s_x[:].rearrange("p (b f) -> p b f", f=H),
                    ps[:, :NB * H].rearrange("p (b f) -> p b f", f=H),
                    s_inn[:].unsqueeze(2).to_broadcast([P, NB, H]),
                    OP.mult))
                dve.tensor_tensor(
                    s_x[:].rearrange("p (b f) -> p b f", f=H),
                    s_x[:].rearrange("p (b f) -> p b f", f=H),
                    s_b[l][:].unsqueeze(1).to_broadcast([P, NB, H]),
                    OP.add).then_inc(S["xd"], 1)
                if l < 2:
                    dve.wait_ge(S["xa"], l + 1)
                    dve.tensor_tensor(
                        s_zall[:].rearrange("p (b f) -> p b f", f=H),
                        s_x[:].rearrange("p (b f) -> p b f", f=H),
                        s_onn[:].unsqueeze(2).to_broadcast([P, NB, H]),
                        OP.mult).then_inc(S["za"], 1)
                    for j in range(NGRP):
                        transform_group(l + 1, j)
            # final
            dve.wait_ge(S["plm"], 1)
            dve.tensor_copy(s_pl[:], PLD[:]).then_inc(S["plc"], 1)
            dve.wait_ge(S["hgl"], 16)
            chain(dve.tensor_scalar_max(s_cnt[:], s_cnt[:], 1.0))
            chain(dve.reciprocal(s_cnt[:], s_cnt[:]))
            dve.tensor_tensor(s_hg[:], s_hg[:], s_cnt[:], OP.mult
                              ).then_inc(S["hgm"], 1)
            dve.wait_ge(S["o2m"], 1)
            dve.tensor_scalar_add(s_o2[:], O2[:], s_be[:]
                                  ).then_inc(S["o2c"], 1)

        # ---------------- PE ----------------
        @block.tensor
        def _(pe):
            pct = dict(t4=0, ym4=0, zc4=0, y4c=0, z4=0)

            def transform_group_pe(tf, j):
                i = j % 2
                r = grp_rows[j]
                gj = tf * NGRP + j
                if tf == 0:
                    pe.wait_ge(S["z4"], j + 1)
                if gj >= 1:
                    pe.wait_ge(S["zc4"], gj)     # TRS slots free
                src = s_z4[i] if tf == 0 else None
                hh = IN_DIM if tf == 0 else H
                for a in range(r):
                    if tf == 0:
                        inap = src[:, a * IN_DIM:(a + 1) * IN_DIM]
                    else:
                        inap = s_zall[:, (4 * j + a) * H:(4 * j + a + 1) * H]
                    pe.transpose(TRS[a][0:hh, :], inap, s_id[:]
                                 ).then_inc(S["t4"], 1)
                pct["t4"] += r
                pe.wait_ge(S["zc4"], gj + 1)
                if gj >= 1:
                    pe.wait_ge(S["y4c"], gj)     # YP4 free
                lhs_t = (s_zT4 if tf == 0 else s_zc4)[i]
                rhs_w = s_W1 if tf == 0 else s_Wn[tf - 1]
                for a in range(r):
                    pe.matmul(YP(a), lhsT=lhs_t[0:hh, a * P:(a + 1) * P],
                              rhs=rhs_w[:], start=True, stop=True
                              ).then_inc(S["ym4"], 1)
                pct["ym4"] += r

            pe.wait_ge(S["ld"], NLD)
            for j in range(NGRP):
                transform_group_pe(0, j)

            for l in range(3):
                if l > 0:
                    pe.wait_ge(S["xd"], l)       # agg consumed by DVE
                for cyc in range(NCYC):
                    gc = l * NCYC + cyc
                    p = gc % NGBUF
                    for j in range(NQ):
                        pe.wait_ge(SG[p][j], 16 * (gc // NGBUF + 1))
                    for hf in range(CYC_TILES // HALF):
                        tbase = cyc * CYC_TILES + hf * HALF
                        hi = tbase + HALF        # tiles [0, hi) needed
                        pe.wait_ge(S["indd"], l * (T // HALF) + hi // HALF)
                        half_last = None
                        for jj in range(HALF):
                            t = tbase + jj
                            blk = int(tile_block[t])
                            if blk < 0:
                                continue
                            ic = l * T + t
                            off = (hf * HALF + jj) * ELEM
                            half_last = pe.matmul(
                                AGG(blk),
                                lhsT=s_ind[ic % IND_RING][:],
                                rhs=s_g[p][:, off:off + H],
                                start=bool(start_f[t]),
                                stop=bool(stop_f[t]))
                        if half_last is not None:
                            half_last.then_inc(S["pe"], 1)
                        else:
                            pe.sem_inc(S["pe"], 1)
                if l < 2:
                    pe.wait_ge(S["za"], l + 1)
                    for j in range(NGRP):
                        transform_group_pe(l + 1, j)
                if l == 2:
                    pe.wait_ge(S["xa"], 3)
                    for b in range(NB):
                        inst = pe.matmul(PLD[:], lhsT=s_x[:, H * b:H * b + H],
                                         rhs=s_poall[:, G * b:G * b + G],
                                         start=(b == 0), stop=(b == NB - 1))
                    inst.then_inc(S["plm"], 1)
            pe.wait_ge(S["hgm"], 1)
            pe.matmul(O2[:], lhsT=s_We[:], rhs=s_hg[:], start=True, stop=True
                      ).then_inc(S["o2m"], 1)

        # ---------------- Pool ----------------
        @block.gpsimd
        def _(g):
            g.wait_ge(S["ld"], NLD)
            g.drain()
            for l in range(3):
                g.wait_ge(S["ywr"], 16 * NGRP * (l + 1))
                g.collective_compute(
                    "AllGather", mybir.AluOpType.bypass,
                    ins=[y_shard[l][:]], outs=[y_full[l][:]],
                    replica_groups=RG).then_inc(S["cc"], 1)
                g.wait_ge(S["cc"], l + 1)
                for cyc in range(NCYC):
                    gc = l * NCYC + cyc
                    p = gc % NGBUF
                    if gc >= NGBUF:
                        g.wait_ge(S["pe"],
                                  (gc - NGBUF + 1) * (CYC_TILES // HALF))
                    for j in range(NQ):
                        k = cyc * NQ + j
                        q = int(inst_pass[k])
                        g.dma_gather(
                            s_g[p][:, j * TPI * ELEM:(j + 1) * TPI * ELEM]
                            .rearrange("p (n e) -> p n e", e=ELEM),
                            y_full[l][q * CHROWS:(q + 1) * CHROWS, :],
                            s_idx[:, k * (INST_IDX // 16):
                                  (k + 1) * (INST_IDX // 16)],
                            INST_IDX, INST_IDX, ELEM,
                            queue_num=j,
                        ).then_inc(SG[p][j], 16)
            g.wait_ge(S["arw"], 16)
            g.collective_compute(
                "AllReduce", mybir.AluOpType.add,
                ins=[ar_in[:]], outs=[ar_out[:]],
                replica_groups=RG).then_inc(S["cc"], 1)

    ctx.close()
    nc.compile()
    return nc


_CACHE = {}


def _run(cfg, inputs, trace=False):
    h = np.asarray(inputs["h"], np.float32)
    src = np.asarray(inputs["src"], np.int64)
    dst = np.asarray(inputs["dst"], np.int64)
    gid = np.asarray(inputs["graph_ids"], np.int64)

    per_core, common, meta = _preprocess(cfg, h, src, dst, gid)

    key = (cfg, meta["T"], meta["NINST"],
           meta["tile_block"].tobytes(), meta["inst_pass"].tobytes())
    if key not in _CACHE:
        _CACHE[key] = _build(cfg, meta)
    nc = _CACHE[key]

    wmap = {
        "W1": inputs["W1"], "W2": inputs["W2"], "W3": inputs["W3"],
        "We": inputs["We"],
        "b1r": np.broadcast_to(np.asarray(inputs["b1"], np.float32),
                               (cfg.P, cfg.H)),
        "b2r": np.broadcast_to(np.asarray(inputs["b2"], np.float32),
                               (cfg.P, cfg.H)),
        "b3r": np.broadcast_to(np.asarray(inputs["b3"], np.float32),
                               (cfg.P, cfg.H)),
        "ber": np.asarray(inputs["be"], np.float32).reshape(cfg.EMB, 1),
    }
    in_maps = []
    for c in range(cfg.CORES):
        m = {}
        for k, v in {**per_core[c], **common, **wmap}.items():
            if k == "idx16":
                m[k] = np.ascontiguousarray(v, dtype=np.int16)
            else:
                m[k] = np.ascontiguousarray(np.asarray(v, np.float32))
        in_maps.append(m)

    from concourse.bass_utils import run_bass_kernel_spmd
    res = run_bass_kernel_spmd(nc, in_maps, core_ids=list(range(cfg.CORES)),
                               trace=trace)
    return np.ascontiguousarray(res.results[0]["outT"].T), res


def kernel(**inputs):
    out, _ = _run(CFG, inputs)
    return out

